# revision 1
# baseline (speedup 1.0000x reference)
"""Trainium2 Bass kernel for nn_Attention_3934190044008.

Multi-head attention with additive bias and sigmoid gating:
  q = (q_x @ w_q) / 8, k = kv_x @ w_k, v = kv_x @ w_v   (8 heads x 64)
  a = softmax(q k^T + bias);  o = a @ v
  o = o * sigmoid(q_x @ w_g + b_g);  out = o @ w_o + b_o

Sharding: 16 (batch, head) pairs over 8 cores -> each core owns one batch
element and 2 heads, produces a partial [2048, 256] output contribution
(o_slice @ w_o rows); host sums the 4 partials per batch and adds b_o.

Device-side layout is "feature on partitions" (transposed): scores are
computed as S^T [k, q] so the softmax denominator rides the AV matmul via a
ones-column appended to V, and softmax-over-k never needs a partition-axis
reduction. All transposes are done on the host (numpy) when building the
per-core input maps.
"""

import os
import sys
import threading
from contextlib import ExitStack

import numpy as np

_REPO = "/opt/trn_rl_repo"
if _REPO not in sys.path and os.path.isdir(_REPO):
    sys.path.insert(0, _REPO)

import concourse.bass as bass  # noqa: E402
import concourse.mybir as mybir  # noqa: E402
import concourse.tile as tile  # noqa: E402
from concourse import bacc  # noqa: E402
from concourse.bass_utils import run_bass_kernel_spmd  # noqa: E402

F32 = mybir.dt.float32
F32R = mybir.dt.float32r

B, SEQ, CQ = 2, 2048, 256
H, DH = 8, 64
HD = H * DH  # 512
N_CORES = 8
HPC = 2  # heads per core

# dtype knobs:
#   QK: "bf16split" (hi/lo bf16 3-product, ~1e-5 on scores), "f32r" (tf32,
#       ~1e-3 on scores), or "f32" (exact, but 4 cycles/row on the PE)
#   AV: "f32r" (tf32) or "f32"
QK_DT = os.environ.get("KRN_QK_DT", "bf16split")
AV_DT = os.environ.get("KRN_AV_DT", "f32r")


def _dt(kind):
    return F32R if kind == "f32r" else F32


def build_nc():
    nc = bacc.Bacc("TRN2", target_bir_lowering=False, debug=False)

    qxT = nc.dram_tensor("qxT", [CQ, SEQ], F32, kind="ExternalInput").ap()
    kvxT = nc.dram_tensor("kvxT", [CQ, SEQ], F32, kind="ExternalInput").ap()
    biasT = nc.dram_tensor("biasT", [HPC, SEQ, SEQ], F32, kind="ExternalInput").ap()
    wq = nc.dram_tensor("wq", [CQ, HPC * DH], F32, kind="ExternalInput").ap()
    wk = nc.dram_tensor("wk", [CQ, HPC * DH], F32, kind="ExternalInput").ap()
    wv = nc.dram_tensor("wv", [CQ, HPC * DH], F32, kind="ExternalInput").ap()
    wg = nc.dram_tensor("wg", [CQ, HPC * DH], F32, kind="ExternalInput").ap()
    bg = nc.dram_tensor("bg", [HPC * DH, 1], F32, kind="ExternalInput").ap()
    wo = nc.dram_tensor("wo", [HPC * DH, CQ], F32, kind="ExternalInput").ap()
    # per-head unnormalized partials + softmax denominators; the division
    # and cross-core summation happen on the host after the gather
    outs_d = [nc.dram_tensor(f"out{h}", [SEQ, CQ], F32, kind="ExternalOutput").ap()
              for h in range(HPC)]
    rs_d = nc.dram_tensor("rs", [1, HPC, SEQ], F32, kind="ExternalOutput").ap()

    NKT = SEQ // 128  # 16 k-tiles
    P = 128

    with tile.TileContext(nc) as tc:
        with ExitStack() as ctx:
            singles = ctx.enter_context(tc.tile_pool(name="singles", bufs=1))

            # ---- resident SBUF tensors ----
            # weights first (tiny, they gate the first projection matmuls);
            # one strided DMA per weight to minimize ~600ns-per-issue
            # sequencer serialization at startup
            w_sbs = {}
            for name, src in (("wk", wk), ("wq", wq), ("wv", wv), ("wg", wg)):
                t = singles.tile([P, 2, P], F32, tag=f"w_{name}")
                eng = nc.sync if name in ("wk", "wv") else nc.scalar
                eng.dma_start(t, src.rearrange("(a p) c -> p a c", p=P))
                w_sbs[name] = t
            bg_sb = singles.tile([P, 1], F32)
            nc.sync.dma_start(bg_sb, bg)
            wo_sb = singles.tile([DH, HPC, CQ], F32)
            nc.scalar.dma_start(wo_sb, wo.rearrange("(h p) c -> p h c", p=DH))

            # inputs as 1 MB halves, K-path first (it gates the first matmuls)
            qxT_sb = singles.tile([P, 2, SEQ], F32)
            kvxT_sb = singles.tile([P, 2, SEQ], F32)
            for a in range(2):
                (nc.sync if a == 0 else nc.scalar).dma_start(
                    kvxT_sb[:, a, :], kvxT[a * P:(a + 1) * P, :])
            for a in range(2):
                (nc.sync if a == 0 else nc.scalar).dma_start(
                    qxT_sb[:, a, :], qxT[a * P:(a + 1) * P, :])

            BF16 = mybir.dt.bfloat16
            if QK_DT == "bf16split":
                # hi/lo bf16 decomposition: S = Kh.Qh + Kl.Qh + Kh.Ql
                # (drops Kl.Ql, ~2^-18 relative on scores).
                KhKl = [singles.tile([P, SEQ], BF16, name=f"KhKl{h}", tag=f"khkl{h}")
                        for h in range(HPC)]  # rows 0-63 Kh, 64-127 Kl
                QhQh = [singles.tile([P, SEQ], BF16, name=f"QhQh{h}", tag=f"qhqh{h}")
                        for h in range(HPC)]  # Qh duplicated on both halves
                Qlo = [singles.tile([DH, SEQ], BF16, name=f"Qlo{h}", tag=f"qlo{h}")
                       for h in range(HPC)]
                KT_sb = QT_sb = None
            else:
                KT_sb = singles.tile([P, SEQ], _dt(QK_DT))   # [2h x 64 d, k]
                QT_sb = singles.tile([P, SEQ], _dt(QK_DT))   # [2h x 64 d, q]
            GT_sb = singles.tile([P, SEQ], F32)  # gate, [2 heads x 64, q]
            V_sb = singles.tile([P, HPC, NKT, DH + 1], _dt(AV_DT))  # [k%128, h, kt, d|1]
            OG_sb = singles.tile([DH, HPC, SEQ], F32)  # (o * g)^T, final lhsT
            rs_sb = singles.tile([1, HPC, SEQ], F32)   # softmax denominators
            ones_col = V_sb[:, :, :, DH:DH + 1]
            if ones_col.dtype == F32R:
                ones_col = ones_col.bitcast(F32)
            nc.vector.memset(ones_col, 1.0)

            # ---- stage B: projections ----
            with tc.tile_pool(name="ppsum", bufs=2, space="PSUM") as ppool, \
                 tc.tile_pool(name="klop", bufs=2) as klop:
                if QK_DT == "bf16split":
                    # Kl staging lives only until the dup-DMA copies it into
                    # KhKl rows 64-127; scoped pool frees its SBUF for the
                    # attention-phase pools
                    Klo_t = [klop.tile([DH, SEQ], BF16, name=f"Klo{h}",
                                       tag="klo") for h in range(HPC)]
                    # per-head M=64 projections so hi/lo tiles land on
                    # partitions 0-63 (DVE cannot move data across partitions)
                    for h in range(HPC):
                        hc = slice(h * DH, (h + 1) * DH)
                        for wt, x_sb, hi, hirow, lo in (
                                (w_sbs["wk"], kvxT_sb, KhKl[h], 0, Klo_t[h]),
                                (w_sbs["wq"], qxT_sb, QhQh[h], 0, Qlo[h])):
                            for tt in range(SEQ // 512):
                                ps = ppool.tile([DH, 512], F32, tag="proj64")
                                nc.tensor.matmul(ps, wt[:, 0, hc],
                                                 x_sb[:, 0, bass.ts(tt, 512)],
                                                 start=True, stop=False)
                                nc.tensor.matmul(ps, wt[:, 1, hc],
                                                 x_sb[:, 1, bass.ts(tt, 512)],
                                                 start=False, stop=True)
                                nc.scalar.copy(hi[0:DH, bass.ts(tt, 512)], ps)
                                nc.vector.tensor_sub(lo[:, bass.ts(tt, 512)], ps,
                                                     hi[0:DH, bass.ts(tt, 512)])
                        # duplicate Qh to rows 64-127; move Kl there too
                        nc.sync.dma_start(QhQh[h][DH:P, :], QhQh[h][0:DH, :])
                        nc.sync.dma_start(KhKl[h][DH:P, :], Klo_t[h][:, :])
                else:
                    for wt, x_sb, dst in ((w_sbs["wq"], qxT_sb, QT_sb),
                                          (w_sbs["wk"], kvxT_sb, KT_sb)):
                        for tt in range(SEQ // 512):
                            ps = ppool.tile([P, 512], F32, tag="proj")
                            nc.tensor.matmul(ps, wt[:, 0, :],
                                             x_sb[:, 0, bass.ts(tt, 512)],
                                             start=True, stop=False)
                            nc.tensor.matmul(ps, wt[:, 1, :],
                                             x_sb[:, 1, bass.ts(tt, 512)],
                                             start=False, stop=True)
                            nc.vector.tensor_copy(dst[:, bass.ts(tt, 512)], ps)
                # gate projection + sigmoid (+ b_g as per-partition bias)
                for tt in range(SEQ // 512):
                    ps = ppool.tile([P, 512], F32, tag="projg")
                    nc.tensor.matmul(ps, w_sbs["wg"][:, 0, :],
                                     qxT_sb[:, 0, bass.ts(tt, 512)],
                                     start=True, stop=False)
                    nc.tensor.matmul(ps, w_sbs["wg"][:, 1, :],
                                     qxT_sb[:, 1, bass.ts(tt, 512)],
                                     start=False, stop=True)
                    nc.scalar.activation(GT_sb[:, bass.ts(tt, 512)], ps,
                                         mybir.ActivationFunctionType.Sigmoid,
                                         bias=bg_sb)
                # V projection: out rows = tokens(k), cols = 2 heads x 64
                for kt in range(NKT):
                    ps = ppool.tile([P, P], F32, tag="vproj")
                    nc.tensor.matmul(ps, kvxT_sb[:, 0, bass.ts(kt, P)], w_sbs["wv"][:, 0, :],
                                     start=True, stop=False)
                    nc.tensor.matmul(ps, kvxT_sb[:, 1, bass.ts(kt, P)], w_sbs["wv"][:, 1, :],
                                     start=False, stop=True)
                    nc.vector.tensor_copy(V_sb[:, 0, kt, 0:DH], ps[:, 0:DH])
                    nc.vector.tensor_copy(V_sb[:, 1, kt, 0:DH], ps[:, DH:2 * DH])

            # ---- stage C: attention ----
            # kt-outer / q-block-inner: one contiguous 1 MB bias DMA per
            # (head, k-tile); both q-block OT accumulators stay live in PSUM
            # (2 x 2 banks) next to the double-buffered S tiles (2 x 2).
            # The softmax epilogue runs on DVE + GpSimd only (reciprocal +
            # partition_broadcast), so PE rolls straight into the next head
            # with no >3.4us idle gap (which would re-throttle the HAM
            # clock to 1.2 GHz).
            QB = 1024
            NQB = SEQ // QB
            with tc.tile_pool(name="otpsum", bufs=2, space="PSUM") as otpool, \
                 tc.tile_pool(name="spsum", bufs=2, space="PSUM") as spool, \
                 tc.tile_pool(name="biasp", bufs=7) as biaspool, \
                 tc.tile_pool(name="sbp", bufs=4) as sbpool, \
                 tc.tile_pool(name="ep", bufs=6) as epool:
                for h in range(HPC):
                    hsl = slice(h * DH, (h + 1) * DH)
                    OTs = [otpool.tile([DH + 1, QB], F32, name=f"OT{h}_{qb}",
                                       tag="ot")
                           for qb in range(NQB)]
                    for kt in range(NKT):
                        bias_sb = biaspool.tile([P, SEQ], F32)
                        # spread bias transfers over three DMA paths (two
                        # HWDGE rings + SWDGE) so they overlap instead of
                        # serializing on one FIFO
                        dma_eng = (nc.sync, nc.scalar, nc.gpsimd)[kt % 3]
                        dma_eng.dma_start(bias_sb, biasT[h, bass.ts(kt, P), :])
                        for qb in range(NQB):
                            q0 = qb * QB
                            S = spool.tile([P, QB], F32, tag="s")
                            if QK_DT == "bf16split":
                                # group same-stationary MMs so the PE keeps
                                # one LDWEIGHTS per weight set (background
                                # double-buffered)
                                for j in range(2):
                                    nc.tensor.matmul(
                                        S[:, bass.ts(j, 512)],
                                        KhKl[h][:, bass.ts(kt, P)],
                                        QhQh[h][:, bass.ds(q0 + j * 512, 512)],
                                        start=True, stop=False)
                                for j in range(2):
                                    nc.tensor.matmul(
                                        S[:, bass.ts(j, 512)],
                                        KhKl[h][0:DH, bass.ts(kt, P)],
                                        Qlo[h][:, bass.ds(q0 + j * 512, 512)],
                                        start=False, stop=True)
                            else:
                                for j in range(2):
                                    nc.tensor.matmul(
                                        S[:, bass.ts(j, 512)],
                                        KT_sb[hsl, bass.ts(kt, P)],
                                        QT_sb[hsl, bass.ds(q0 + j * 512, 512)],
                                        start=True, stop=True)
                            SB = sbpool.tile([P, QB], F32, tag="SB")
                            nc.vector.tensor_add(SB, S, bias_sb[:, bass.ds(q0, QB)])
                            E = epool.tile([P, QB], _dt(AV_DT))
                            nc.scalar.activation(E, SB, mybir.ActivationFunctionType.Exp)
                            for j in range(2):
                                nc.tensor.matmul(
                                    OTs[qb][:, bass.ts(j, 512)],
                                    V_sb[:, h, kt, :],
                                    E[:, bass.ts(j, 512)],
                                    start=(kt == 0), stop=(kt == NKT - 1))
                    # gate (unnormalized) and stash the exp-sum row; the
                    # softmax division happens on the host
                    for qb in range(NQB):
                        q0 = qb * QB
                        OT = OTs[qb]
                        # on the last head the exp-sum copy rides ACT so
                        # the DVE epilogue chain stays under the ~3.4us HAM
                        # re-throttle window before the output projections
                        if h == HPC - 1:
                            nc.scalar.copy(rs_sb[:, h, bass.ds(q0, QB)],
                                           OT[DH:DH + 1, :])
                        else:
                            nc.vector.tensor_copy(rs_sb[:, h, bass.ds(q0, QB)],
                                                  OT[DH:DH + 1, :])
                        nc.vector.tensor_mul(OG_sb[:, h, bass.ds(q0, QB)],
                                             GT_sb[hsl, bass.ds(q0, QB)],
                                             OT[0:DH, :])


                # ---- stage D: per-head output projections (partials) ----
                # inside the attention pool scope, with PSUM riding the
                # S-pool slots: no pool-close barrier, so head 0's finals
                # (ready since mid-kernel) start the moment an S slot frees
                # after the last exp, covering head 1's epilogue on DVE and
                # keeping the PE clock warm into the tail.
                for h in range(HPC):
                    for tt in range(SEQ // P):
                        ps = spool.tile([P, CQ], F32, tag="s", name="fin_ps")
                        nc.tensor.matmul(ps, OG_sb[:, h, bass.ts(tt, P)],
                                         wo_sb[:, h, :], start=True, stop=True)
                        o_sb = sbpool.tile([P, CQ], F32, tag="SB",
                                           name="fin_osb")
                        nc.vector.tensor_copy(o_sb, ps)
                        eng = nc.sync if tt % 2 == 0 else nc.scalar
                        eng.dma_start(outs_d[h][bass.ts(tt, P), :], o_sb)

            nc.sync.dma_start(rs_d, rs_sb)

    nc.compile()
    return nc


_NC = None
_NC_LOCK = threading.Lock()


def _get_nc():
    global _NC
    with _NC_LOCK:
        if _NC is None:
            _NC = build_nc()
        return _NC


def make_in_maps(q_x, kv_x, bias, w_q, w_k, w_v, w_g, b_g, w_o, b_o):
    del b_o  # added on the host after the gather
    q_x = np.asarray(q_x, dtype=np.float32)
    kv_x = np.asarray(kv_x, dtype=np.float32)
    bias = np.asarray(bias, dtype=np.float32)
    w_q = np.asarray(w_q, dtype=np.float32) * np.float32(0.125)  # fold 1/sqrt(64)
    w_k = np.asarray(w_k, dtype=np.float32)
    w_v = np.asarray(w_v, dtype=np.float32)
    w_g = np.asarray(w_g, dtype=np.float32)
    b_g = np.asarray(b_g, dtype=np.float32)
    w_o = np.asarray(w_o, dtype=np.float32)

    in_maps = []
    for c in range(N_CORES):
        b = c // (N_CORES // B)
        h0 = HPC * (c % (N_CORES // B))
        cols = slice(h0 * DH, (h0 + HPC) * DH)
        in_maps.append({
            "qxT": np.ascontiguousarray(q_x[b].T),
            "kvxT": np.ascontiguousarray(kv_x[b].T),
            "biasT": np.ascontiguousarray(bias[b, h0:h0 + HPC].swapaxes(1, 2)),
            "wq": np.ascontiguousarray(w_q[:, cols]),
            "wk": np.ascontiguousarray(w_k[:, cols]),
            "wv": np.ascontiguousarray(w_v[:, cols]),
            "wg": np.ascontiguousarray(w_g[:, cols]),
            "bg": np.ascontiguousarray(b_g[cols].reshape(HPC * DH, 1)),
            "wo": np.ascontiguousarray(w_o[cols, :]),
        })
    return in_maps


def gather_output(results, b_o):
    full = np.zeros((B, SEQ, CQ), dtype=np.float32)
    for c in range(N_CORES):
        b = c // (N_CORES // B)
        rs = results[c]["rs"][0]
        for h in range(HPC):
            full[b] += results[c][f"out{h}"] / rs[h][:, None]
    full += np.asarray(b_o, dtype=np.float32)
    return full


def kernel(**inputs):
    nc = _get_nc()
    in_maps = make_in_maps(**inputs)
    res = run_bass_kernel_spmd(nc, in_maps, core_ids=list(range(N_CORES)))
    return gather_output(res.results, inputs["b_o"])



# revision 4
# speedup vs baseline: 1.2232x; 1.2232x over previous
"""Trainium2 Bass kernel for nn_Attention_3934190044008.

Multi-head attention with additive bias and sigmoid gating:
  q = (q_x @ w_q) / 8, k = kv_x @ w_k, v = kv_x @ w_v   (8 heads x 64)
  a = softmax(q k^T + bias);  o = a @ v
  o = o * sigmoid(q_x @ w_g + b_g);  out = o @ w_o + b_o

Sharding: 16 (batch, head) pairs over 8 cores -> each core owns one batch
element and 2 heads, produces per-head unnormalized partial outputs
(o*g)^T @ w_o plus the softmax denominators; the host divides by the
denominators, sums the partials per batch, and adds b_o.

Key layout/engine choices (v2):
- Scores computed transposed, S^T [k, q], so softmax-over-k needs no
  partition reduction; the denominator rides the AV matmul via a ones
  column in V.
- exp(s + b) = exp(s - 3) * exp(b - 3): the host ships exp(bias - 3) in
  fp16, ACT exps the raw scores straight out of PSUM (with its free
  affine bias for the -3 shift), and the "bias add" becomes a 16-bit
  SBUF multiply on DVE that hits the 2x perf mode.  This avoids the
  1x-rate f32 PSUM adds that dominated DVE in the baseline.
- All matmuls (QK, AV, projections, out-proj) run fp16 at 1 cyc/row
  instead of bf16split (2 passes) / f32 (4 cyc/row).
- The gate is computed as tanh (same ACT table set as exp -> one table
  load): sigmoid(z) = 0.5 (1 + tanh(z/2)), with the 0.5 folded into w_o
  on the host and (1 + t) fused into the epilogue scalar_tensor_tensor.
"""

import os
import sys
import threading
from contextlib import ExitStack

import numpy as np

_REPO = "/opt/trn_rl_repo"
if _REPO not in sys.path and os.path.isdir(_REPO):
    sys.path.insert(0, _REPO)

import concourse.bass as bass  # noqa: E402
import concourse.mybir as mybir  # noqa: E402
import concourse.tile as tile  # noqa: E402
from concourse import bacc  # noqa: E402
from concourse.bass_utils import run_bass_kernel_spmd  # noqa: E402

F32 = mybir.dt.float32
F16 = mybir.dt.float16

B, SEQ, CQ = 2, 2048, 256
H, DH = 8, 64
HD = H * DH  # 512
N_CORES = 8
HPC = 2  # heads per core
NKT = SEQ // 128  # 16 k-tiles
P = 128
QB = 1024  # q block (one S/E tile)
NQB = SEQ // QB
SHIFT = 3.0  # exp(s+b) = exp(s-SHIFT)*exp(b-SHIFT); cancels in softmax

# fraction of E-multiplies routed to gpsimd (0..NKT); tune from trace
GPS_MULT_EVERY = int(os.environ.get("KRN_GPS_MULT_EVERY", "0"))


def build_nc():
    nc = bacc.Bacc("TRN2", target_bir_lowering=False, debug=False)

    qxT = nc.dram_tensor("qxT", [CQ, SEQ], F16, kind="ExternalInput").ap()
    kvxT = nc.dram_tensor("kvxT", [CQ, SEQ], F16, kind="ExternalInput").ap()
    ebT = nc.dram_tensor("ebT", [HPC, SEQ, SEQ], F16, kind="ExternalInput").ap()
    wq = nc.dram_tensor("wq", [CQ, HPC * DH], F16, kind="ExternalInput").ap()
    wk = nc.dram_tensor("wk", [CQ, HPC * DH], F16, kind="ExternalInput").ap()
    wv = nc.dram_tensor("wv", [CQ, HPC * DH], F16, kind="ExternalInput").ap()
    wg = nc.dram_tensor("wg", [CQ, HPC * DH], F16, kind="ExternalInput").ap()
    bgh = nc.dram_tensor("bgh", [DH, HPC], F32, kind="ExternalInput").ap()
    wo = nc.dram_tensor("wo", [HPC * DH, CQ], F16, kind="ExternalInput").ap()
    # per-head unnormalized out partials, heads side by side: [tok, 2*256]
    out_d = nc.dram_tensor("out", [SEQ, HPC * CQ], F16, kind="ExternalOutput").ap()
    rs_d = nc.dram_tensor("rs", [1, HPC, SEQ], F16, kind="ExternalOutput").ap()

    with tile.TileContext(nc) as tc:
        with ExitStack() as ctx:
            singles = ctx.enter_context(tc.tile_pool(name="singles", bufs=1))
            biaspool = ctx.enter_context(tc.tile_pool(name="biasp", bufs=6))

            # ---- resident SBUF tensors ----
            # weights via SWDGE (gpsimd) so the SP ring is free for inputs
            w_sbs = {}
            for name, src in (("wk", wk), ("wq", wq), ("wv", wv), ("wg", wg)):
                t = singles.tile([P, 2, P], F16, tag=f"w_{name}")
                nc.gpsimd.dma_start(t, src.rearrange("(a p) c -> p a c", p=P))
                w_sbs[name] = t
            wo_sb = singles.tile([DH, HPC, CQ], F16)
            nc.gpsimd.dma_start(wo_sb, wo.rearrange("(h p) c -> p h c", p=DH))
            bgh_sb = singles.tile([DH, HPC], F32)
            nc.gpsimd.dma_start(bgh_sb, bgh)

            # inputs as 0.5 MB halves on SP, K-path first (gates first matmuls)
            qxT_sb = singles.tile([P, 2, SEQ], F16)
            kvxT_sb = singles.tile([P, 2, SEQ], F16)
            for a in range(2):
                nc.sync.dma_start(kvxT_sb[:, a, :], kvxT[a * P:(a + 1) * P, :])
            for a in range(2):
                nc.sync.dma_start(qxT_sb[:, a, :], qxT[a * P:(a + 1) * P, :])

            KT_sb = singles.tile([P, SEQ], F16)   # [2h x 64 d, k]
            QT_sb = singles.tile([P, SEQ], F16)   # [2h x 64 d, q]
            t_sb = singles.tile([DH + 1, HPC, SEQ], F16)  # tanh gate, row 64 = 0
            V_sb = singles.tile([P, HPC, NKT, DH + 1], F16)  # [k%128, h, kt, d|1]
            OG_sb = singles.tile([DH + 1, HPC, SEQ], F16)  # (1+t)*OT; row 64 = rs
            out_sb = singles.tile([P, NKT, HPC * CQ], F16)
            shift_sb = singles.tile([P, 1], F32)
            nc.vector.memset(V_sb[:, :, :, DH:DH + 1], 1.0)
            nc.vector.memset(t_sb[DH:DH + 1, :, :], 0.0)
            nc.vector.memset(shift_sb, -SHIFT)

            # ---- stage B: projections (all fp16, 1 cyc/row) ----
            with tc.tile_pool(name="ppsum", bufs=2, space="PSUM") as ppool:
                for wt, x_sb, dst in ((w_sbs["wk"], kvxT_sb, KT_sb),
                                      (w_sbs["wq"], qxT_sb, QT_sb)):
                    for tt in range(SEQ // 512):
                        ps = ppool.tile([P, 512], F32, tag="proj")
                        nc.tensor.matmul(ps, wt[:, 0, :],
                                         x_sb[:, 0, bass.ts(tt, 512)],
                                         start=True, stop=False)
                        nc.tensor.matmul(ps, wt[:, 1, :],
                                         x_sb[:, 1, bass.ts(tt, 512)],
                                         start=False, stop=True)
                        nc.vector.tensor_copy(dst[:, bass.ts(tt, 512)], ps)
                # V projection: out rows = tokens(k), cols = 2 heads x 64
                for kt in range(NKT):
                    ps = ppool.tile([P, P], F32, tag="vproj")
                    nc.tensor.matmul(ps, kvxT_sb[:, 0, bass.ts(kt, P)],
                                     w_sbs["wv"][:, 0, :], start=True, stop=False)
                    nc.tensor.matmul(ps, kvxT_sb[:, 1, bass.ts(kt, P)],
                                     w_sbs["wv"][:, 1, :], start=False, stop=True)
                    nc.vector.tensor_copy(
                        V_sb[:, :, kt, 0:DH],
                        ps.rearrange("p (h d) -> p h d", h=HPC))
                # gate projection per head (so each head's features land on
                # partitions 0-63, matching OT's partition range) + tanh on
                # ACT: sigmoid(z) = 0.5(1+tanh(z/2)), 0.5 folded into wo
                for h in range(HPC):
                    hc = slice(h * DH, (h + 1) * DH)
                    for tt in range(SEQ // 512):
                        ps = ppool.tile([DH, 512], F32, tag="gproj")
                        nc.tensor.matmul(ps, w_sbs["wg"][:, 0, hc],
                                         qxT_sb[:, 0, bass.ts(tt, 512)],
                                         start=True, stop=False)
                        nc.tensor.matmul(ps, w_sbs["wg"][:, 1, hc],
                                         qxT_sb[:, 1, bass.ts(tt, 512)],
                                         start=False, stop=True)
                        nc.scalar.activation(t_sb[0:DH, h, bass.ts(tt, 512)], ps,
                                             mybir.ActivationFunctionType.Tanh,
                                             bias=bgh_sb[:, h:h + 1], scale=0.5)

            # ---- stage C: attention ----
            with tc.tile_pool(name="spsum", bufs=2, space="PSUM") as spool, \
                 tc.tile_pool(name="otpsum", bufs=2, space="PSUM") as otpool, \
                 tc.tile_pool(name="e0p", bufs=3) as e0pool, \
                 tc.tile_pool(name="ep", bufs=3) as epool:
                for h in range(HPC):
                    hsl = slice(h * DH, (h + 1) * DH)
                    OTs = [otpool.tile([DH + 1, QB], F32, name=f"OT{h}_{qb}",
                                       tag="ot")
                           for qb in range(NQB)]
                    for kt in range(NKT):
                        eb_sb = biaspool.tile([P, SEQ], F16)
                        nc.sync.dma_start(eb_sb, ebT[h, bass.ts(kt, P), :])
                        Ss, Es = [], []
                        for qb in range(NQB):
                            S = spool.tile([P, QB], F32, tag="s")
                            for j in range(2):
                                nc.tensor.matmul(
                                    S[:, bass.ts(j, 512)],
                                    KT_sb[hsl, bass.ts(kt, P)],
                                    QT_sb[hsl, bass.ds(qb * QB + j * 512, 512)],
                                    start=True, stop=True)
                            Ss.append(S)
                        for qb in range(NQB):
                            E0 = e0pool.tile([P, QB], F16, tag="e0")
                            nc.scalar.activation(E0, Ss[qb],
                                                 mybir.ActivationFunctionType.Exp,
                                                 bias=shift_sb)
                            E = epool.tile([P, QB], F16, tag="e")
                            meng = (nc.gpsimd if GPS_MULT_EVERY and
                                    kt % GPS_MULT_EVERY == GPS_MULT_EVERY - 1
                                    else nc.vector)
                            meng.tensor_mul(E, E0, eb_sb[:, bass.ds(qb * QB, QB)])
                            Es.append(E)
                        for qb in range(NQB):
                            for j in range(2):
                                nc.tensor.matmul(
                                    OTs[qb][:, bass.ts(j, 512)],
                                    V_sb[:, h, kt, :],
                                    Es[qb][:, bass.ts(j, 512)],
                                    start=(kt == 0), stop=(kt == NKT - 1))
                    # epilogue: OG = (1 + t) * OT; row 64 (t=0) passes rs
                    for qb in range(NQB):
                        qsl = bass.ds(qb * QB, QB)
                        nc.vector.scalar_tensor_tensor(
                            OG_sb[:, h, qsl], t_sb[:, h, qsl], 1.0, OTs[qb],
                            op0=mybir.AluOpType.add, op1=mybir.AluOpType.mult)

                # ---- stage D: per-head output projections (partials) ----
                # rides spool's PSUM slots: no pool-close barrier
                for tt in range(SEQ // P):
                    ps = spool.tile([P, HPC * CQ], F32, tag="s", name="fin_ps")
                    for h in range(HPC):
                        nc.tensor.matmul(ps[:, bass.ds(h * CQ, CQ)],
                                         OG_sb[0:DH, h, bass.ts(tt, P)],
                                         wo_sb[:, h, :], start=True, stop=True)
                    nc.vector.tensor_copy(out_sb[:, tt, :], ps)
                    if tt % 4 == 3:
                        nc.sync.dma_start(
                            out_d.rearrange("(t p) c -> p t c", p=P)[
                                :, tt - 3:tt + 1, :],
                            out_sb[:, tt - 3:tt + 1, :])

            nc.sync.dma_start(rs_d, OG_sb[DH:DH + 1, :, :])

    nc.compile()
    return nc


_NC = None
_NC_LOCK = threading.Lock()


def _get_nc():
    global _NC
    with _NC_LOCK:
        if _NC is None:
            _NC = build_nc()
        return _NC


def make_in_maps(q_x, kv_x, bias, w_q, w_k, w_v, w_g, b_g, w_o, b_o):
    del b_o  # added on the host after the gather
    q_x = np.asarray(q_x, dtype=np.float32)
    kv_x = np.asarray(kv_x, dtype=np.float32)
    bias = np.asarray(bias, dtype=np.float32)
    w_q = np.asarray(w_q, dtype=np.float32) * np.float32(0.125)  # fold 1/sqrt(64)
    w_k = np.asarray(w_k, dtype=np.float32)
    w_v = np.asarray(w_v, dtype=np.float32)
    w_g = np.asarray(w_g, dtype=np.float32)
    b_g = np.asarray(b_g, dtype=np.float32)
    w_o = np.asarray(w_o, dtype=np.float32) * np.float32(0.5)  # tanh gate trick

    qxT = {}
    kvxT = {}
    for b in range(B):
        qxT[b] = np.ascontiguousarray(q_x[b].T.astype(np.float16))
        kvxT[b] = np.ascontiguousarray(kv_x[b].T.astype(np.float16))

    in_maps = []
    for c in range(N_CORES):
        b = c // (N_CORES // B)
        h0 = HPC * (c % (N_CORES // B))
        cols = slice(h0 * DH, (h0 + HPC) * DH)
        ebT = np.exp(bias[b, h0:h0 + HPC].swapaxes(1, 2) - np.float32(SHIFT))
        in_maps.append({
            "qxT": qxT[b],
            "kvxT": kvxT[b],
            "ebT": np.ascontiguousarray(ebT.astype(np.float16)),
            "wq": np.ascontiguousarray(w_q[:, cols].astype(np.float16)),
            "wk": np.ascontiguousarray(w_k[:, cols].astype(np.float16)),
            "wv": np.ascontiguousarray(w_v[:, cols].astype(np.float16)),
            "wg": np.ascontiguousarray(w_g[:, cols].astype(np.float16)),
            "bgh": np.ascontiguousarray(
                (0.5 * b_g[cols]).reshape(HPC, DH).T.astype(np.float32)),
            "wo": np.ascontiguousarray(w_o[cols, :].astype(np.float16)),
        })
    return in_maps


def gather_output(results, b_o):
    full = np.zeros((B, SEQ, CQ), dtype=np.float32)
    for c in range(N_CORES):
        b = c // (N_CORES // B)
        out = results[c]["out"].astype(np.float32)
        rs = results[c]["rs"][0].astype(np.float32)
        for h in range(HPC):
            full[b] += out[:, h * CQ:(h + 1) * CQ] / rs[h][:, None]
    full += np.asarray(b_o, dtype=np.float32)
    return full


def kernel(**inputs):
    nc = _get_nc()
    in_maps = make_in_maps(**inputs)
    res = run_bass_kernel_spmd(nc, in_maps, core_ids=list(range(N_CORES)))
    return gather_output(res.results, inputs["b_o"])


# revision 7
# speedup vs baseline: 1.3951x; 1.1406x over previous
"""Trainium2 Bass kernel for nn_Attention_3934190044008.

Multi-head attention with additive bias and sigmoid gating:
  q = (q_x @ w_q) / 8, k = kv_x @ w_k, v = kv_x @ w_v   (8 heads x 64)
  a = softmax(q k^T + bias);  o = a @ v
  o = o * sigmoid(q_x @ w_g + b_g);  out = o @ w_o + b_o

Sharding: 16 (batch, head) pairs over 8 cores -> each core owns one batch
element and 2 heads, produces per-head unnormalized partial outputs
(o*g)^T @ w_o plus the softmax denominators; the host divides by the
denominators (1/rs commutes through the linear w_o), sums the partials
per batch, and adds b_o.

Key layout/engine choices (v3):
- Scores computed transposed, S^T [k, q]; softmax-over-k needs no
  partition reduction; denominator rides the AV matmul via a ones column
  in V.
- Two bias paths, chosen per k-tile (INJECT_EVERY knob):
  * mult path: host ships exp(bias-3) fp16; ACT does E0=exp(s-3) straight
    from PSUM; DVE multiplies E=E0*eb in 16-bit 2x mode.
  * inject path: host ships raw bias fp16; the PE adds it into the score
    PSUM via an identity-stationary matmul and ACT does E=exp(s+b-6)
    directly.  This is deliberate PE "filler": the HAM clock gate only
    stays at 2.4 GHz while the PE is nearly gap-free, so the PE must be
    the (slightly) slowest engine in the attention loop.
- All matmuls fp16 at 1 cyc/row.  Gate via tanh (same ACT table set as
  exp -> one table load): sigmoid(z) = 0.5 (1 + tanh(z/2)), 0.5 folded
  into w_o, (1 + t) fused via scalar_tensor_tensor; t row 64 = 0 lets
  the denominator ride through the same op.
- Head 0's output projection is retired inside head 1's attention loop
  (more PE filler); only head 1's runs in the tail.
"""

import os
import sys
import threading
from contextlib import ExitStack

import numpy as np

_REPO = "/opt/trn_rl_repo"
if _REPO not in sys.path and os.path.isdir(_REPO):
    sys.path.insert(0, _REPO)

import concourse.bass as bass  # noqa: E402
import concourse.mybir as mybir  # noqa: E402
import concourse.tile as tile  # noqa: E402
from concourse import bacc  # noqa: E402
from concourse.bass_utils import run_bass_kernel_spmd  # noqa: E402

F32 = mybir.dt.float32
F16 = mybir.dt.float16

B, SEQ, CQ = 2, 2048, 256
H, DH = 8, 64
HD = H * DH  # 512
N_CORES = 8
HPC = 2  # heads per core
NKT = SEQ // 128  # 16 k-tiles
P = 128
QB = 1024  # q block (one S/E tile)
NQB = SEQ // QB
SHIFT = 3.0  # total softmax shift is 2*SHIFT; cancels in the division

# kt % INJECT_EVERY == INJECT_EVERY-1 -> PE-inject bias path (0 = never)
INJECT_EVERY = int(os.environ.get("KRN_INJECT_EVERY", "2"))


def _is_inject(kt):
    return INJECT_EVERY > 0 and kt % INJECT_EVERY == INJECT_EVERY - 1


def build_nc():
    nc = bacc.Bacc("TRN2", target_bir_lowering=False, debug=False)

    qxT = nc.dram_tensor("qxT", [CQ, SEQ], F16, kind="ExternalInput").ap()
    kvxT = nc.dram_tensor("kvxT", [CQ, SEQ], F16, kind="ExternalInput").ap()
    ebT = nc.dram_tensor("ebT", [HPC, SEQ, SEQ], F16, kind="ExternalInput").ap()
    wq = nc.dram_tensor("wq", [CQ, HPC * DH], F16, kind="ExternalInput").ap()
    wk = nc.dram_tensor("wk", [CQ, HPC * DH], F16, kind="ExternalInput").ap()
    wv = nc.dram_tensor("wv", [CQ, HPC * DH], F16, kind="ExternalInput").ap()
    wg = nc.dram_tensor("wg", [CQ, HPC * DH], F16, kind="ExternalInput").ap()
    bgh = nc.dram_tensor("bgh", [DH, HPC], F32, kind="ExternalInput").ap()
    wo = nc.dram_tensor("wo", [HPC * DH, CQ], F16, kind="ExternalInput").ap()
    ident = nc.dram_tensor("ident", [P, P], F16, kind="ExternalInput").ap()
    # per-head unnormalized out partials, heads side by side: [tok, 2*256]
    out_d = nc.dram_tensor("out", [SEQ, HPC * CQ], F16, kind="ExternalOutput").ap()
    rs_d = nc.dram_tensor("rs", [1, HPC, SEQ], F16, kind="ExternalOutput").ap()

    with tile.TileContext(nc) as tc:
        with ExitStack() as ctx:
            singles = ctx.enter_context(tc.tile_pool(name="singles", bufs=1))
            biaspool = ctx.enter_context(tc.tile_pool(name="biasp", bufs=6))

            # weights via SWDGE (gpsimd) so the SP ring is free for inputs
            w_sbs = {}
            for name, src in (("wk", wk), ("wq", wq), ("wv", wv), ("wg", wg)):
                t = singles.tile([P, 2, P], F16, tag=f"w_{name}")
                nc.gpsimd.dma_start(t, src.rearrange("(a p) c -> p a c", p=P))
                w_sbs[name] = t
            wo_sb = singles.tile([DH, HPC, CQ], F16)
            nc.gpsimd.dma_start(wo_sb, wo.rearrange("(h p) c -> p h c", p=DH))
            bgh_sb = singles.tile([DH, HPC], F32)
            nc.gpsimd.dma_start(bgh_sb, bgh)
            I_sb = singles.tile([P, P], F16)
            nc.gpsimd.dma_start(I_sb, ident)

            # inputs as 0.5 MB halves on SP, K-path first (gates first matmuls)
            qxT_sb = singles.tile([P, 2, SEQ], F16)
            kvxT_sb = singles.tile([P, 2, SEQ], F16)
            for a in range(2):
                nc.sync.dma_start(kvxT_sb[:, a, :], kvxT[a * P:(a + 1) * P, :])
            for a in range(2):
                nc.sync.dma_start(qxT_sb[:, a, :], qxT[a * P:(a + 1) * P, :])

            KT_sb = singles.tile([P, SEQ], F16)   # [2h x 64 d, k]
            QT_sb = singles.tile([P, SEQ], F16)   # [2h x 64 d, q]
            t_sb = singles.tile([DH + 1, HPC, SEQ], F16)  # tanh gate; row 64 = 0
            V_sb = singles.tile([P, HPC, NKT, DH + 1], F16)  # [k%128, h, kt, d|1]
            OG_sb = singles.tile([DH + 1, HPC, SEQ], F16)  # (1+t)*OT; row 64 = rs
            out_sb = singles.tile([P, NKT, HPC * CQ], F16)
            shift3_sb = singles.tile([P, 1], F32)
            shift6_sb = singles.tile([P, 1], F32)
            nc.vector.memset(V_sb[:, :, :, DH:DH + 1], 1.0)
            nc.vector.memset(t_sb[DH:DH + 1, :, :], 0.0)
            nc.vector.memset(shift3_sb, -SHIFT)
            nc.vector.memset(shift6_sb, -2.0 * SHIFT)

            # pre-issue the first bias tiles so the attention fill phase
            # never stalls on DMA (a mostly-idle 3.4us window re-throttles
            # the PE clock to 1.2 GHz and it does not recover)
            eb_tiles = {}
            for kt in range(5):
                eb = biaspool.tile([P, SEQ], F16)
                nc.sync.dma_start(eb, ebT[0, bass.ts(kt, P), :])
                eb_tiles[(0, kt)] = eb

            # ---- stage B: projections as one dense back-to-back MM block
            # (HAM warmup burst).  ACT fills its idle prologue with the gate
            # tanh chunks; exp instructions queue behind them but only wait
            # on their own S tiles.
            with tc.tile_pool(name="ppsum", bufs=2, space="PSUM") as ppool:
                kq = [(w_sbs["wk"], kvxT_sb, KT_sb), (w_sbs["wq"], qxT_sb, QT_sb)]
                for tt in range(SEQ // 512):
                    for wt, x_sb, dst in kq:
                        ps = ppool.tile([P, 512], F32, tag="proj")
                        nc.tensor.matmul(ps, wt[:, 0, :],
                                         x_sb[:, 0, bass.ts(tt, 512)],
                                         start=True, stop=False)
                        nc.tensor.matmul(ps, wt[:, 1, :],
                                         x_sb[:, 1, bass.ts(tt, 512)],
                                         start=False, stop=True)
                        nc.vector.tensor_copy(dst[:, bass.ts(tt, 512)], ps)
                # V projection: out rows = tokens(k), cols = 2 heads x 64
                for kt in range(NKT):
                    ps = ppool.tile([P, P], F32, tag="vproj")
                    nc.tensor.matmul(ps, kvxT_sb[:, 0, bass.ts(kt, P)],
                                     w_sbs["wv"][:, 0, :], start=True, stop=False)
                    nc.tensor.matmul(ps, kvxT_sb[:, 1, bass.ts(kt, P)],
                                     w_sbs["wv"][:, 1, :], start=False, stop=True)
                    nc.vector.tensor_copy(
                        V_sb[:, :, kt, 0:DH],
                        ps.rearrange("p (h d) -> p h d", h=HPC))
                # gate per head (features on partitions 0-63 to match OT)
                for h in range(HPC):
                    hc = slice(h * DH, (h + 1) * DH)
                    for tt in range(SEQ // 512):
                        ps = ppool.tile([DH, 512], F32, tag="gproj")
                        nc.tensor.matmul(ps, w_sbs["wg"][:, 0, hc],
                                         qxT_sb[:, 0, bass.ts(tt, 512)],
                                         start=True, stop=False)
                        nc.tensor.matmul(ps, w_sbs["wg"][:, 1, hc],
                                         qxT_sb[:, 1, bass.ts(tt, 512)],
                                         start=False, stop=True)
                        nc.scalar.activation(t_sb[0:DH, h, bass.ts(tt, 512)], ps,
                                             mybir.ActivationFunctionType.Tanh,
                                             bias=bgh_sb[:, h:h + 1], scale=0.5)

            # ---- stage C: attention ----
            with tc.tile_pool(name="spsum", bufs=2, space="PSUM") as spool, \
                 tc.tile_pool(name="otpsum", bufs=2, space="PSUM") as otpool, \
                 tc.tile_pool(name="e0p", bufs=4) as e0pool, \
                 tc.tile_pool(name="ep", bufs=4) as epool:

                def out_proj(tt, h):
                    # unnormalized per-head partial: (OG_h)^T @ (0.5 wo_h);
                    # rides spool's slots (no separate PSUM budget)
                    ps = spool.tile([P, CQ], F32, tag="s", name="fin_ps")
                    nc.tensor.matmul(ps, OG_sb[0:DH, h, bass.ts(tt, P)],
                                     wo_sb[:, h, :], start=True, stop=True)
                    nc.vector.tensor_copy(out_sb[:, tt, bass.ds(h * CQ, CQ)], ps)

                for h in range(HPC):
                    hsl = slice(h * DH, (h + 1) * DH)
                    OTs = [otpool.tile([DH + 1, QB], F32, name=f"OT{h}_{qb}",
                                       tag="ot")
                           for qb in range(NQB)]
                    for kt in range(NKT):
                        inj = _is_inject(kt)
                        if (h, kt) in eb_tiles:
                            eb_sb = eb_tiles.pop((h, kt))
                        else:
                            eb_sb = biaspool.tile([P, SEQ], F16)
                            nc.sync.dma_start(eb_sb, ebT[h, bass.ts(kt, P), :])
                        Ss, Es = [], []
                        for qb in range(NQB):
                            S = spool.tile([P, QB], F32, tag="s")
                            for j in range(2):
                                nc.tensor.matmul(
                                    S[:, bass.ts(j, 512)],
                                    KT_sb[hsl, bass.ts(kt, P)],
                                    QT_sb[hsl, bass.ds(qb * QB + j * 512, 512)],
                                    start=True, stop=not inj)
                            Ss.append(S)
                        if inj:
                            # PE adds the bias into the score PSUM (keeps the
                            # PE the pacing engine so HAM stays at 2.4 GHz)
                            for qb in range(NQB):
                                for j in range(2):
                                    nc.tensor.matmul(
                                        Ss[qb][:, bass.ts(j, 512)],
                                        I_sb,
                                        eb_sb[:, bass.ds(qb * QB + j * 512, 512)],
                                        start=False, stop=True)
                        # head 1 retires head 0's out-proj as extra PE filler
                        if h == 1:
                            out_proj(kt, 0)
                        for qb in range(NQB):
                            E = e0pool.tile([P, QB], F16, tag="e0")
                            nc.scalar.activation(
                                E, Ss[qb], mybir.ActivationFunctionType.Exp,
                                bias=shift6_sb if inj else shift3_sb)
                            if not inj:
                                E2 = epool.tile([P, QB], F16, tag="e")
                                nc.vector.tensor_mul(
                                    E2, E, eb_sb[:, bass.ds(qb * QB, QB)])
                                E = E2
                            Es.append(E)
                        for qb in range(NQB):
                            for j in range(2):
                                nc.tensor.matmul(
                                    OTs[qb][:, bass.ts(j, 512)],
                                    V_sb[:, h, kt, :],
                                    Es[qb][:, bass.ts(j, 512)],
                                    start=(kt == 0), stop=(kt == NKT - 1))
                    # epilogue: OG = (1 + t) * OT; row 64 (t=0) passes rs
                    for qb in range(NQB):
                        qsl = bass.ds(qb * QB, QB)
                        nc.vector.scalar_tensor_tensor(
                            OG_sb[:, h, qsl], t_sb[:, h, qsl], 1.0, OTs[qb],
                            op0=mybir.AluOpType.add, op1=mybir.AluOpType.mult)

                # ---- stage D: head-1 output projection tail ----
                for tt in range(SEQ // P):
                    out_proj(tt, 1)
                    if tt % 4 == 3:
                        nc.sync.dma_start(
                            out_d.rearrange("(t p) c -> p t c", p=P)[
                                :, tt - 3:tt + 1, :],
                            out_sb[:, tt - 3:tt + 1, :])

            nc.sync.dma_start(rs_d, OG_sb[DH:DH + 1, :, :])

    nc.compile()
    return nc


_NC = None
_NC_LOCK = threading.Lock()


def _get_nc():
    global _NC
    with _NC_LOCK:
        if _NC is None:
            _NC = build_nc()
        return _NC


def make_in_maps(q_x, kv_x, bias, w_q, w_k, w_v, w_g, b_g, w_o, b_o):
    del b_o  # added on the host after the gather
    q_x = np.asarray(q_x, dtype=np.float32)
    kv_x = np.asarray(kv_x, dtype=np.float32)
    bias = np.asarray(bias, dtype=np.float32)
    w_q = np.asarray(w_q, dtype=np.float32) * np.float32(0.125)  # fold 1/sqrt(64)
    w_k = np.asarray(w_k, dtype=np.float32)
    w_v = np.asarray(w_v, dtype=np.float32)
    w_g = np.asarray(w_g, dtype=np.float32)
    b_g = np.asarray(b_g, dtype=np.float32)
    w_o = np.asarray(w_o, dtype=np.float32) * np.float32(0.5)  # tanh gate trick

    qxT = {}
    kvxT = {}
    for b in range(B):
        qxT[b] = np.ascontiguousarray(q_x[b].T.astype(np.float16))
        kvxT[b] = np.ascontiguousarray(kv_x[b].T.astype(np.float16))
    ident = np.eye(P, dtype=np.float16)

    in_maps = []
    for c in range(N_CORES):
        b = c // (N_CORES // B)
        h0 = HPC * (c % (N_CORES // B))
        cols = slice(h0 * DH, (h0 + HPC) * DH)
        # per-kt rows: raw bias for inject k-tiles, exp(bias-3) for mult
        bT = np.ascontiguousarray(bias[b, h0:h0 + HPC].swapaxes(1, 2))
        ebT = np.empty((HPC, SEQ, SEQ), dtype=np.float16)
        for kt in range(NKT):
            rows = slice(kt * P, (kt + 1) * P)
            if _is_inject(kt):
                ebT[:, rows, :] = bT[:, rows, :].astype(np.float16)
            else:
                ebT[:, rows, :] = np.exp(
                    bT[:, rows, :] - np.float32(SHIFT)).astype(np.float16)
        in_maps.append({
            "qxT": qxT[b],
            "kvxT": kvxT[b],
            "ebT": ebT,
            "wq": np.ascontiguousarray(w_q[:, cols].astype(np.float16)),
            "wk": np.ascontiguousarray(w_k[:, cols].astype(np.float16)),
            "wv": np.ascontiguousarray(w_v[:, cols].astype(np.float16)),
            "wg": np.ascontiguousarray(w_g[:, cols].astype(np.float16)),
            "bgh": np.ascontiguousarray(
                (0.5 * b_g[cols]).reshape(HPC, DH).T.astype(np.float32)),
            "wo": np.ascontiguousarray(w_o[cols, :].astype(np.float16)),
            "ident": ident,
        })
    return in_maps


def gather_output(results, b_o):
    full = np.zeros((B, SEQ, CQ), dtype=np.float32)
    for c in range(N_CORES):
        b = c // (N_CORES // B)
        out = results[c]["out"].astype(np.float32)
        rs = results[c]["rs"][0].astype(np.float32)
        for h in range(HPC):
            full[b] += out[:, h * CQ:(h + 1) * CQ] / rs[h][:, None]
    full += np.asarray(b_o, dtype=np.float32)
    return full


def kernel(**inputs):
    nc = _get_nc()
    in_maps = make_in_maps(**inputs)
    res = run_bass_kernel_spmd(nc, in_maps, core_ids=list(range(N_CORES)))
    return gather_output(res.results, inputs["b_o"])


# revision 8
# speedup vs baseline: 1.5424x; 1.1056x over previous
"""Trainium2 Bass kernel for nn_Attention_3934190044008.

Multi-head attention with additive bias and sigmoid gating:
  q = (q_x @ w_q) / 8, k = kv_x @ w_k, v = kv_x @ w_v   (8 heads x 64)
  a = softmax(q k^T + bias);  o = a @ v
  o = o * sigmoid(q_x @ w_g + b_g);  out = o @ w_o + b_o

Sharding: 16 (batch, head) pairs over 8 cores -> each core owns one batch
element and 2 heads, produces per-head unnormalized partial outputs
(o*g)^T @ w_o plus the softmax denominators; the host divides by the
denominators (1/rs commutes through the linear w_o), sums the partials
per batch, and adds b_o.

Key layout/engine choices (v4):
- Scores computed transposed, S^T [k, q]; softmax-over-k needs no
  partition reduction; denominator rides the AV matmul via a ones column
  in V.
- Two bias paths, chosen per k-tile (INJECT_EVERY knob):
  * mult path: host ships exp(bias-3) fp16; ACT does E0=exp(s-3) straight
    from PSUM; DVE multiplies E=E0*eb in 16-bit 2x mode.
  * inject path: host ships raw bias fp16; the PE adds it into the score
    PSUM via an identity-stationary matmul and ACT does E=exp(s+b-6)
    directly.  This doubles as PE "filler": the HAM clock gate only stays
    at 2.4 GHz while the PE is nearly gap-free, so the PE should be the
    (slightly) slowest engine in the attention loop.
- AV matmuls run one k-tile behind QK (software pipelining) so the PE
  never waits on the exp/mult chain of the current tile.
- All matmuls fp16 at 1 cyc/row (issue rate ~216ns per 512-row MM warm).
  Gate via tanh (same ACT table set as exp -> one table load):
  sigmoid(z) = 0.5 (1 + tanh(z/2)), 0.5 folded into w_o, (1 + t) fused
  via scalar_tensor_tensor; t row 64 = 0 lets the denominator ride
  through the same op.
- Head 0's output projection + output DMA retire inside head 1's
  attention loop; only head 1's runs in the tail.
"""

import os
import sys
import threading
from contextlib import ExitStack

import numpy as np

_REPO = "/opt/trn_rl_repo"
if _REPO not in sys.path and os.path.isdir(_REPO):
    sys.path.insert(0, _REPO)

import concourse.bass as bass  # noqa: E402
import concourse.mybir as mybir  # noqa: E402
import concourse.tile as tile  # noqa: E402
from concourse import bacc  # noqa: E402
from concourse.bass_utils import run_bass_kernel_spmd  # noqa: E402

F32 = mybir.dt.float32
F16 = mybir.dt.float16

B, SEQ, CQ = 2, 2048, 256
H, DH = 8, 64
HD = H * DH  # 512
N_CORES = 8
HPC = 2  # heads per core
NKT = SEQ // 128  # 16 k-tiles
P = 128
QB = 1024  # q block (one S/E tile)
NQB = SEQ // QB
SHIFT = 3.0  # total softmax shift is 2*SHIFT; cancels in the division

# kt % INJECT_EVERY == INJECT_EVERY-1 -> PE-inject bias path (0 = never)
INJECT_EVERY = int(os.environ.get("KRN_INJECT_EVERY", "2"))


def _is_inject(kt):
    return INJECT_EVERY > 0 and kt % INJECT_EVERY == INJECT_EVERY - 1


def build_nc():
    nc = bacc.Bacc("TRN2", target_bir_lowering=False, debug=False)

    qxT = nc.dram_tensor("qxT", [CQ, SEQ], F16, kind="ExternalInput").ap()
    kvxT = nc.dram_tensor("kvxT", [CQ, SEQ], F16, kind="ExternalInput").ap()
    ebT = nc.dram_tensor("ebT", [HPC, SEQ, SEQ], F16, kind="ExternalInput").ap()
    wq = nc.dram_tensor("wq", [CQ, HPC * DH], F16, kind="ExternalInput").ap()
    wk = nc.dram_tensor("wk", [CQ, HPC * DH], F16, kind="ExternalInput").ap()
    wv = nc.dram_tensor("wv", [CQ, HPC * DH], F16, kind="ExternalInput").ap()
    wg = nc.dram_tensor("wg", [CQ, HPC * DH], F16, kind="ExternalInput").ap()
    bgh = nc.dram_tensor("bgh", [DH, HPC], F32, kind="ExternalInput").ap()
    wo = nc.dram_tensor("wo", [HPC * DH, CQ], F16, kind="ExternalInput").ap()
    ident = nc.dram_tensor("ident", [P, P], F16, kind="ExternalInput").ap()
    out0_d = nc.dram_tensor("out0", [SEQ, CQ], F16, kind="ExternalOutput").ap()
    out1_d = nc.dram_tensor("out1", [SEQ, CQ], F16, kind="ExternalOutput").ap()
    rs_d = nc.dram_tensor("rs", [1, HPC, SEQ], F16, kind="ExternalOutput").ap()

    with tile.TileContext(nc) as tc:
        with ExitStack() as ctx:
            singles = ctx.enter_context(tc.tile_pool(name="singles", bufs=1))
            biaspool = ctx.enter_context(tc.tile_pool(name="biasp", bufs=6))

            # K/Q weights first on the fast SP ring (they gate the first
            # projection matmuls), then the inputs; everything else rides
            # the gpsimd SWDGE queue.
            w_sbs = {}
            for name, src in (("wk", wk), ("wq", wq)):
                t = singles.tile([P, 2, P], F16, tag=f"w_{name}")
                nc.sync.dma_start(t, src.rearrange("(a p) c -> p a c", p=P))
                w_sbs[name] = t
            qxT_sb = singles.tile([P, 2, SEQ], F16)
            kvxT_sb = singles.tile([P, 2, SEQ], F16)
            for a in range(2):
                nc.sync.dma_start(kvxT_sb[:, a, :], kvxT[a * P:(a + 1) * P, :])
            for a in range(2):
                nc.sync.dma_start(qxT_sb[:, a, :], qxT[a * P:(a + 1) * P, :])
            for name, src in (("wv", wv), ("wg", wg)):
                t = singles.tile([P, 2, P], F16, tag=f"w_{name}")
                nc.gpsimd.dma_start(t, src.rearrange("(a p) c -> p a c", p=P))
                w_sbs[name] = t
            wo_sb = singles.tile([DH, HPC, CQ], F16)
            nc.gpsimd.dma_start(wo_sb, wo.rearrange("(h p) c -> p h c", p=DH))
            bgh_sb = singles.tile([DH, HPC], F32)
            nc.gpsimd.dma_start(bgh_sb, bgh)
            I_sb = singles.tile([P, P], F16)
            nc.gpsimd.dma_start(I_sb, ident)

            KT_sb = singles.tile([P, SEQ], F16)   # [2h x 64 d, k]
            QT_sb = singles.tile([P, SEQ], F16)   # [2h x 64 d, q]
            t_sb = singles.tile([DH + 1, HPC, SEQ], F16)  # tanh gate; row 64 = 0
            V_sb = singles.tile([P, HPC, NKT, DH + 1], F16)  # [k%128, h, kt, d|1]
            OG_sb = singles.tile([DH + 1, HPC, SEQ], F16)  # (1+t)*OT; row 64 = rs
            out_sb = singles.tile([P, NKT, HPC, CQ], F16)
            shift3_sb = singles.tile([P, 1], F32)
            shift6_sb = singles.tile([P, 1], F32)
            nc.vector.memset(V_sb[:, :, :, DH:DH + 1], 1.0)
            nc.vector.memset(t_sb[DH:DH + 1, :, :], 0.0)
            nc.vector.memset(shift3_sb, -SHIFT)
            nc.vector.memset(shift6_sb, -2.0 * SHIFT)

            # pre-issue the first bias tiles so the attention fill phase
            # never stalls on DMA
            eb_tiles = {}
            for kt in range(5):
                eb = biaspool.tile([P, SEQ], F16)
                nc.sync.dma_start(eb, ebT[0, bass.ts(kt, P), :])
                eb_tiles[(0, kt)] = eb

            # ---- stage B: projections as one dense back-to-back MM block
            # (HAM warmup burst).  ACT fills its idle prologue with the
            # gate tanh chunks.
            with tc.tile_pool(name="ppsum", bufs=2, space="PSUM") as ppool:
                for wt, x_sb, dst in ((w_sbs["wk"], kvxT_sb, KT_sb),
                                      (w_sbs["wq"], qxT_sb, QT_sb)):
                    for tt in range(SEQ // 512):
                        ps = ppool.tile([P, 512], F32, tag="proj")
                        nc.tensor.matmul(ps, wt[:, 0, :],
                                         x_sb[:, 0, bass.ts(tt, 512)],
                                         start=True, stop=False)
                        nc.tensor.matmul(ps, wt[:, 1, :],
                                         x_sb[:, 1, bass.ts(tt, 512)],
                                         start=False, stop=True)
                        nc.vector.tensor_copy(dst[:, bass.ts(tt, 512)], ps)
                # V projection: out rows = tokens(k), cols = 2 heads x 64
                for kt in range(NKT):
                    ps = ppool.tile([P, P], F32, tag="vproj")
                    nc.tensor.matmul(ps, kvxT_sb[:, 0, bass.ts(kt, P)],
                                     w_sbs["wv"][:, 0, :], start=True, stop=False)
                    nc.tensor.matmul(ps, kvxT_sb[:, 1, bass.ts(kt, P)],
                                     w_sbs["wv"][:, 1, :], start=False, stop=True)
                    nc.vector.tensor_copy(
                        V_sb[:, :, kt, 0:DH],
                        ps.rearrange("p (h d) -> p h d", h=HPC))
                # gate per head (features on partitions 0-63 to match OT)
                for h in range(HPC):
                    hc = slice(h * DH, (h + 1) * DH)
                    for tt in range(SEQ // 512):
                        ps = ppool.tile([DH, 512], F32, tag="gproj")
                        nc.tensor.matmul(ps, w_sbs["wg"][:, 0, hc],
                                         qxT_sb[:, 0, bass.ts(tt, 512)],
                                         start=True, stop=False)
                        nc.tensor.matmul(ps, w_sbs["wg"][:, 1, hc],
                                         qxT_sb[:, 1, bass.ts(tt, 512)],
                                         start=False, stop=True)
                        nc.scalar.activation(t_sb[0:DH, h, bass.ts(tt, 512)], ps,
                                             mybir.ActivationFunctionType.Tanh,
                                             bias=bgh_sb[:, h:h + 1], scale=0.5)

            # ---- stage C: attention ----
            with tc.tile_pool(name="spsum", bufs=2, space="PSUM") as spool, \
                 tc.tile_pool(name="otpsum", bufs=2, space="PSUM") as otpool, \
                 tc.tile_pool(name="e0p", bufs=6) as e0pool, \
                 tc.tile_pool(name="ep", bufs=6) as epool:

                def out_proj(tt, h):
                    # unnormalized per-head partial: (OG_h)^T @ (0.5 wo_h);
                    # rides spool's slots (no separate PSUM budget)
                    ps = spool.tile([P, CQ], F32, tag="s", name="fin_ps")
                    nc.tensor.matmul(ps, OG_sb[0:DH, h, bass.ts(tt, P)],
                                     wo_sb[:, h, :], start=True, stop=True)
                    nc.vector.tensor_copy(out_sb[:, tt, h, :], ps)

                def out_dma(tt_hi, h):
                    # DMA 4 token-tiles of head h once their copies landed
                    dst = (out0_d, out1_d)[h].rearrange("(t p) c -> p t c", p=P)
                    nc.sync.dma_start(dst[:, tt_hi - 3:tt_hi + 1, :],
                                      out_sb[:, tt_hi - 3:tt_hi + 1, h, :])

                for h in range(HPC):
                    hsl = slice(h * DH, (h + 1) * DH)
                    OTs = [otpool.tile([DH + 1, QB], F32, name=f"OT{h}_{qb}",
                                       tag="ot")
                           for qb in range(NQB)]
                    Es_prev = None

                    def av(kt, Es):
                        for qb in range(NQB):
                            for j in range(2):
                                nc.tensor.matmul(
                                    OTs[qb][:, bass.ts(j, 512)],
                                    V_sb[:, h, kt, :],
                                    Es[qb][:, bass.ts(j, 512)],
                                    start=(kt == 0), stop=(kt == NKT - 1))

                    for kt in range(NKT):
                        inj = _is_inject(kt)
                        if (h, kt) in eb_tiles:
                            eb_sb = eb_tiles.pop((h, kt))
                        else:
                            eb_sb = biaspool.tile([P, SEQ], F16)
                            nc.sync.dma_start(eb_sb, ebT[h, bass.ts(kt, P), :])
                        Ss, Es = [], []
                        for qb in range(NQB):
                            S = spool.tile([P, QB], F32, tag="s")
                            for j in range(2):
                                nc.tensor.matmul(
                                    S[:, bass.ts(j, 512)],
                                    KT_sb[hsl, bass.ts(kt, P)],
                                    QT_sb[hsl, bass.ds(qb * QB + j * 512, 512)],
                                    start=True, stop=not inj)
                            Ss.append(S)
                        if inj:
                            # PE adds the bias into the score PSUM
                            for qb in range(NQB):
                                for j in range(2):
                                    nc.tensor.matmul(
                                        Ss[qb][:, bass.ts(j, 512)],
                                        I_sb,
                                        eb_sb[:, bass.ds(qb * QB + j * 512, 512)],
                                        start=False, stop=True)
                        # head 1 retires head 0's out-proj + DMA as PE filler
                        # (kt-2 lag so the h0 epilogue never stalls the PE)
                        if h == 1 and kt >= 2:
                            out_proj(kt - 2, 0)
                            if kt % 4 == 1 and kt >= 5:
                                out_dma(kt - 2, 0)
                        # AV for the previous k-tile (its E chain is done)
                        if Es_prev is not None:
                            av(kt - 1, Es_prev)
                        for qb in range(NQB):
                            E = e0pool.tile([P, QB], F16, tag="e0")
                            nc.scalar.activation(
                                E, Ss[qb], mybir.ActivationFunctionType.Exp,
                                bias=shift6_sb if inj else shift3_sb)
                            if not inj:
                                E2 = epool.tile([P, QB], F16, tag="e")
                                nc.vector.tensor_mul(
                                    E2, E, eb_sb[:, bass.ds(qb * QB, QB)])
                                E = E2
                            Es.append(E)
                        Es_prev = Es
                    av(NKT - 1, Es_prev)
                    # epilogue: OG = (1 + t) * OT; row 64 (t=0) passes rs
                    for qb in range(NQB):
                        qsl = bass.ds(qb * QB, QB)
                        nc.vector.scalar_tensor_tensor(
                            OG_sb[:, h, qsl], t_sb[:, h, qsl], 1.0, OTs[qb],
                            op0=mybir.AluOpType.add, op1=mybir.AluOpType.mult)

                # ---- stage D: tail ----
                for tt in (NKT - 2, NKT - 1):
                    out_proj(tt, 0)
                out_dma(NKT - 1, 0)
                for tt in range(SEQ // P):
                    out_proj(tt, 1)
                    if tt % 4 == 3:
                        out_dma(tt, 1)

            nc.sync.dma_start(rs_d, OG_sb[DH:DH + 1, :, :])

    nc.compile()
    return nc


_NC = None
_NC_LOCK = threading.Lock()


def _get_nc():
    global _NC
    with _NC_LOCK:
        if _NC is None:
            _NC = build_nc()
        return _NC


def make_in_maps(q_x, kv_x, bias, w_q, w_k, w_v, w_g, b_g, w_o, b_o):
    del b_o  # added on the host after the gather
    q_x = np.asarray(q_x, dtype=np.float32)
    kv_x = np.asarray(kv_x, dtype=np.float32)
    bias = np.asarray(bias, dtype=np.float32)
    w_q = np.asarray(w_q, dtype=np.float32) * np.float32(0.125)  # fold 1/sqrt(64)
    w_k = np.asarray(w_k, dtype=np.float32)
    w_v = np.asarray(w_v, dtype=np.float32)
    w_g = np.asarray(w_g, dtype=np.float32)
    b_g = np.asarray(b_g, dtype=np.float32)
    w_o = np.asarray(w_o, dtype=np.float32) * np.float32(0.5)  # tanh gate trick

    qxT = {}
    kvxT = {}
    for b in range(B):
        qxT[b] = np.ascontiguousarray(q_x[b].T.astype(np.float16))
        kvxT[b] = np.ascontiguousarray(kv_x[b].T.astype(np.float16))
    ident = np.eye(P, dtype=np.float16)

    in_maps = []
    for c in range(N_CORES):
        b = c // (N_CORES // B)
        h0 = HPC * (c % (N_CORES // B))
        cols = slice(h0 * DH, (h0 + HPC) * DH)
        # per-kt rows: raw bias for inject k-tiles, exp(bias-3) for mult
        bT = np.ascontiguousarray(bias[b, h0:h0 + HPC].swapaxes(1, 2))
        ebT = np.empty((HPC, SEQ, SEQ), dtype=np.float16)
        for kt in range(NKT):
            rows = slice(kt * P, (kt + 1) * P)
            if _is_inject(kt):
                ebT[:, rows, :] = bT[:, rows, :].astype(np.float16)
            else:
                ebT[:, rows, :] = np.exp(
                    bT[:, rows, :] - np.float32(SHIFT)).astype(np.float16)
        in_maps.append({
            "qxT": qxT[b],
            "kvxT": kvxT[b],
            "ebT": ebT,
            "wq": np.ascontiguousarray(w_q[:, cols].astype(np.float16)),
            "wk": np.ascontiguousarray(w_k[:, cols].astype(np.float16)),
            "wv": np.ascontiguousarray(w_v[:, cols].astype(np.float16)),
            "wg": np.ascontiguousarray(w_g[:, cols].astype(np.float16)),
            "bgh": np.ascontiguousarray(
                (0.5 * b_g[cols]).reshape(HPC, DH).T.astype(np.float32)),
            "wo": np.ascontiguousarray(w_o[cols, :].astype(np.float16)),
            "ident": ident,
        })
    return in_maps


def gather_output(results, b_o):
    full = np.zeros((B, SEQ, CQ), dtype=np.float32)
    for c in range(N_CORES):
        b = c // (N_CORES // B)
        rs = results[c]["rs"][0].astype(np.float32)
        for h in range(HPC):
            out = results[c][f"out{h}"].astype(np.float32)
            full[b] += out / rs[h][:, None]
    full += np.asarray(b_o, dtype=np.float32)
    return full


def kernel(**inputs):
    nc = _get_nc()
    in_maps = make_in_maps(**inputs)
    res = run_bass_kernel_spmd(nc, in_maps, core_ids=list(range(N_CORES)))
    return gather_output(res.results, inputs["b_o"])
